# revision 18
# baseline (speedup 1.0000x reference)
"""Causal multi-head self-attention with RoPE on 8 TRN2 NeuronCores.

Problem (hardcoded): B=2, S=2048, D=1024, H=16, d_k=64, fp32 I/O.

Sharding (data + tensor parallel, per the head-group hint):
  core c in 0..7 -> batch b = c//4, head group g = c%4 (4 heads = 256 dims).
  Wq/Wk/Wv split column-wise (by output head dims), Wo split row-wise.
  Each core computes a partial [S, D] output (bf16); the host sums the 4
  partials per batch in fp32 (row-parallel unshard).

Device kernel layout (engine-balanced software pipeline):
  - Q,K computed transposed [e, s] so scores need no transposes. Weight rows
    are permuted per head (evens then odds) so RoPE becomes full-width
    elementwise ops plus one 32-row-block permutation matmul on the PE.
  - Three overlapped phases, ordered so the PE always has dense work:
      A: per-512-chunk projections of Q/K (head pair 0) + V interleaved with
         the input DMA stream (ACT does PSUM evictions; DVE does RoPE).
      B: attention for head pair 0 with pair-1 projection chunks emitted as
         fillers inside each column, covering ACT-exp and normalize latency.
      C: attention for pair 1, with the previous column's output
         projection + output DMA emitted as in-column fillers.
  - Scores [sk 128, 2(par), sq] per sk-tile in one 2-bank PSUM tile; the two
    per-head score matmuls are adjacent K=64 row-group tiles (tile_position
    auto-derived (0,0)/(64,0)) so they pack concurrently in the PE array; ONE
    wide ACT exp per sk-tile covers both heads. Causally dead sq ranges are
    never computed (matmul, exp and PV all trimmed to [c*128:]). PV runs one
    sk-tile behind scores so column-boundary PSUM WARs never block the queue.
  - Softmax denominator comes free as a 65th output row of the P@V matmul via
    a ones-column appended to V. No max-subtraction (scores bounded; fp32
    PSUM).  All matmul inputs bf16 (full PE rate), fp32 PSUM accumulation.
"""

import numpy as np
import ml_dtypes

B, S, D = 2, 2048, 1024
H, DK = 16, 64
HPC = 4          # heads per core
E = HPC * DK     # 256 output dims per core
P = 128
KS = D // P      # 8 contraction subtiles
SQT = 512        # sq column width
NJ = S // SQT    # 4 sq columns
NSK = S // P     # 16 sk tiles
VW = 96          # per-head V slot: [ones, 31 pad, v0..63]
DK1 = DK + 1
BF = ml_dtypes.bfloat16

_CACHE = {}


def _build_nc():
    import concourse.bacc as bacc
    import concourse.mybir as mybir
    import concourse.tile as tile
    from contextlib import ExitStack

    bf = mybir.dt.bfloat16
    f32 = mybir.dt.float32
    Exp = mybir.ActivationFunctionType.Exp

    nc = bacc.Bacc("TRN2", target_bir_lowering=False)

    xT = nc.dram_tensor("xT", [D, S], bf, kind="ExternalInput")
    wq = nc.dram_tensor("wq", [D, E], bf, kind="ExternalInput")
    wk = nc.dram_tensor("wk", [D, E], bf, kind="ExternalInput")
    wv = nc.dram_tensor("wv", [D, E], bf, kind="ExternalInput")
    wo = nc.dram_tensor("wo", [E, D], bf, kind="ExternalInput")
    cs = nc.dram_tensor("cs", [P, S], bf, kind="ExternalInput")
    sn = nc.dram_tensor("sn", [P, S], bf, kind="ExternalInput")
    tri = nc.dram_tensor("tri", [P, P], bf, kind="ExternalInput")
    swp = nc.dram_tensor("swp", [P, P], bf, kind="ExternalInput")
    out = nc.dram_tensor("out", [S, D], bf, kind="ExternalOutput")

    with tile.TileContext(nc) as tc, ExitStack() as ctx:
        const = ctx.enter_context(tc.tile_pool(name="const", bufs=1))
        work = ctx.enter_context(tc.tile_pool(name="work", bufs=2))
        pexp_pool = ctx.enter_context(tc.tile_pool(name="pexpp", bufs=6))
        mm = ctx.enter_context(tc.tile_pool(name="mm", bufs=2, space="PSUM"))
        stp_pool = ctx.enter_context(tc.tile_pool(name="stp", bufs=2, space="PSUM"))
        pv_pool = ctx.enter_context(tc.tile_pool(name="pvp", bufs=1, space="PSUM"))

        # ---- input DMAs, ordered so phase A can start as early as
        # possible: Wq, x chunk 0, Wk, Wv (V matmuls directly follow q/k of
        # each chunk in the PE queue), RoPE/mask constants, remaining x
        # chunks, Wo last (needed only in phase C).
        xTv = xT.rearrange("(ks p) s -> p ks s", p=P)
        wq_sb = const.tile([P, KS, E], bf, tag="wq")
        nc.sync.dma_start(wq_sb[:], wq.rearrange("(ks p) e -> p ks e", p=P))
        xss = []
        for st in range(NJ):
            xc = const.tile([P, KS, SQT], bf, tag=f"xs{st}", name=f"xs{st}")
            xss.append(xc)
        nc.sync.dma_start(xss[0][:], xTv[:, :, 0:SQT])
        wk_sb = const.tile([P, KS, E], bf, tag="wk")
        nc.sync.dma_start(wk_sb[:], wk.rearrange("(ks p) e -> p ks e", p=P))
        wv_sb = const.tile([P, KS, E], bf, tag="wv")
        nc.sync.dma_start(wv_sb[:], wv.rearrange("(ks p) e -> p ks e", p=P))
        swp_sb = const.tile([P, P], bf, tag="swp")
        nc.sync.dma_start(swp_sb[:], swp[:])
        nc.sync.dma_start(xss[1][:], xTv[:, :, SQT:2 * SQT])
        cs_sb = const.tile([P, S], bf, tag="cs")
        nc.sync.dma_start(cs_sb[:], cs[:])
        sn_sb = const.tile([P, S], bf, tag="sn")
        nc.sync.dma_start(sn_sb[:], sn[:])
        tri_sb = const.tile([P, P], bf, tag="tri")
        nc.sync.dma_start(tri_sb[:], tri[:])
        nc.sync.dma_start(xss[2][:], xTv[:, :, 2 * SQT:3 * SQT])
        nc.sync.dma_start(xss[3][:], xTv[:, :, 3 * SQT:4 * SQT])
        wo_sb = const.tile([P, 2, D], bf, tag="wo")
        nc.sync.dma_start(wo_sb[:], wo.rearrange("(ks p) e -> p ks e", p=P))

        qts = [const.tile([P, S], bf, tag=f"qt{eb}", name=f"qt{eb}") for eb in range(2)]
        kts = [const.tile([P, S], bf, tag=f"kt{eb}", name=f"kt{eb}") for eb in range(2)]
        # V per head padded to 96 cols: [ones, 31 pad, v0..63]. The leading
        # ones column puts the softmax denominator on PSUM partition 0, where
        # DVE can take the reciprocal straight off PSUM (no ACT involvement,
        # which would head-of-line block the exp queue). The pad keeps the
        # value rows on legal PSUM partition bases (32/64); 96-wide weights
        # keep the per-matmul LDWEIGHTS 25% cheaper than a 128-padded slot.
        vaug = const.tile([P, NSK, HPC * VW], bf, tag="vaug")
        vaug4 = vaug.rearrange("p t (h e) -> p t h e", h=HPC)
        nc.vector.memset(vaug4[:, :, :, 1:32], 0.0)
        nc.vector.memset(vaug4[:, :, :, 0], 1.0)
        # normalized attention values, laid out as Wo lhsT [d%128, d//128, sq]
        vals = const.tile([P, 2, S], bf, tag="vals")

        def proj_chunk(w_sb, eb, st, dst, act_evict, c0=0, clen=SQT):
            """One chunk of (W.T @ x.T) for e-block eb, with RoPE applied.
            act_evict: evict main PSUM via ACT (phase A) or DVE (phase B)."""
            sl = slice(st * SQT + c0, st * SQT + c0 + clen)
            ps = mm.tile([P, SQT], f32, tag="mm", name="ps")[:, 0:clen]
            for ks in range(KS):
                nc.tensor.matmul(
                    ps,
                    lhsT=w_sb[:, ks, eb * P:(eb + 1) * P],
                    rhs=xss[st][:, ks, c0:c0 + clen],
                    start=(ks == 0), stop=(ks == KS - 1),
                )
            q0 = work.tile([P, SQT], bf, tag="q0", name="q0")[:, 0:clen]
            if act_evict:
                nc.scalar.copy(out=q0, in_=ps)
            else:
                nc.vector.tensor_copy(out=q0, in_=ps)
            psw = mm.tile([P, SQT], f32, tag="mm", name="psw")[:, 0:clen]
            nc.tensor.matmul(psw, lhsT=swp_sb[:], rhs=q0,
                             start=True, stop=True)
            t = work.tile([P, SQT], bf, tag="ropet", name="t")[:, 0:clen]
            nc.vector.tensor_mul(out=t, in0=q0, in1=cs_sb[:, sl])
            sw = work.tile([P, SQT], bf, tag="ropesw", name="sw")[:, 0:clen]
            nc.vector.tensor_mul(out=sw, in0=psw, in1=sn_sb[:, sl])
            nc.vector.tensor_add(out=dst[:, sl], in0=t, in1=sw)

        # ---- phase A: Q/K pair 0 + V, per x-chunk, following the DMA stream
        for st in range(NJ):
            proj_chunk(wq_sb, 0, st, qts[0], act_evict=True)
            proj_chunk(wk_sb, 0, st, kts[0], act_evict=True)
            # V for all 4 heads, natural [s, e] layout, into vaug slots
            for s4 in range(4):
                sst = st * 4 + s4
                ps = mm.tile([P, SQT], f32, tag="mm")
                pv256 = ps[:, 0:E]
                for ks in range(KS):
                    nc.tensor.matmul(
                        pv256,
                        lhsT=xss[st][:, ks, s4 * P:(s4 + 1) * P],
                        rhs=wv_sb[:, ks, :],
                        start=(ks == 0), stop=(ks == KS - 1),
                    )
                nc.scalar.copy(
                    out=vaug4[:, sst, :, 32:VW],
                    in_=pv256.rearrange("p (h e) -> p h e", h=HPC),
                )

        def attention_col(pair, j, fillers):
            """Attention for head pair `pair`, sq column j. `fillers` are
            independent emission pieces spread between sk-tiles so the PE
            queue never drains on ACT/normalize latency."""
            jsl = slice(j * SQT, (j + 1) * SQT)
            last_i = 4 * j + 3
            n_i = last_i + 1
            fill_at = {}
            for k in range(len(fillers)):
                pos = (n_i * (k + 1)) // (len(fillers) + 1)
                fill_at.setdefault(pos, []).append(k)
            pvts = [pv_pool.tile([P, SQT], f32, tag=f"pv{par}", name=f"pv{par}")
                    for par in range(2)]
            pending = None  # PV runs one sk-tile behind scores/exp

            def emit_pv(item):
                i, pexp, off = item
                for par in range(2):
                    hl = 2 * pair + par
                    nc.tensor.matmul(
                        pvts[par][0:VW, off:SQT],
                        lhsT=vaug[:, i, hl * VW:(hl + 1) * VW],
                        rhs=pexp[:, par, off:SQT],
                        start=(i == 0), stop=(i == last_i),
                    )

            for i in range(n_i):
                c = i - 4 * j
                off = c * P if c > 0 else 0
                stp = stp_pool.tile([P, 2, SQT], f32, tag="stp", name="stp")
                for par in range(2):
                    nc.tensor.matmul(
                        stp[:, par, off:SQT],
                        lhsT=kts[pair][64 * par:64 * par + 64, i * P:(i + 1) * P],
                        rhs=qts[pair][64 * par:64 * par + 64,
                                      j * SQT + off:(j + 1) * SQT],
                        start=True, stop=True,
                    )
                pexp = pexp_pool.tile([P, 2, SQT], bf, tag="pexp", name="pexp")
                nc.scalar.activation(out=pexp[:, :, off:SQT],
                                     in_=stp[:, :, off:SQT], func=Exp)
                if c >= 0:  # diagonal subtile: triangular mask
                    for par in range(2):
                        dsl = pexp[:, par, off:off + P]
                        nc.vector.tensor_mul(out=dsl, in0=dsl, in1=tri_sb[:])
                if pending is not None:
                    emit_pv(pending)
                pending = (i, pexp, off)
                for k in fill_at.get(i, ()):
                    fillers[k]()
            emit_pv(pending)
            # normalize by softmax denominator (PSUM row 0 of pvt): fast
            # reciprocal straight off PSUM on DVE, broadcast, scale, then an
            # SBUF->SBUF DMA into the vals row block for this head.
            for par in range(2):
                rsb = work.tile([1, SQT], f32, tag="rsb")
                nc.vector.reciprocal_approx_fast(out=rsb[0:1, :],
                                                 in_=pvts[par][0:1, :])
                rb = work.tile([VW, SQT], f32, tag="rb")
                nc.gpsimd.partition_broadcast(rb[0:VW, :], rsb[0:1, :],
                                              channels=VW)
                # value rows live at 32:96; >32-partition PSUM reads must
                # start at 0/64, so scale in two 32-row halves.
                stg = work.tile([P, SQT], bf, tag="stg")
                nc.vector.tensor_mul(out=stg[32:64, :], in0=pvts[par][32:64, :],
                                     in1=rb[32:64, :])
                nc.vector.tensor_mul(out=stg[64:VW, :], in0=pvts[par][64:VW, :],
                                     in1=rb[64:VW, :])
                dst = vals[64 * par:64 * par + 64, pair, jsl]
                nc.sync.dma_start(out=dst, in_=stg[32:VW, :])

        # ---- phase B: attention pair 0, pair-1 projections as fillers
        for j in range(NJ):
            attention_col(0, j, [
                (lambda st=j: proj_chunk(wq_sb, 1, st, qts[1], act_evict=False)),
                (lambda st=j: proj_chunk(wk_sb, 1, st, kts[1], act_evict=False)),
            ])

        # ---- phase C: attention pair 1, previous column's out-projection
        # emitted as fillers of the current column
        def outproj_tile(sq):
            # out[sq*128:(sq+1)*128, :] = vals[:, :, sq-tile].T @ woT
            for n2 in range(2):
                ps = mm.tile([P, SQT], f32, tag="mm")
                for ks2 in range(2):
                    nc.tensor.matmul(
                        ps[:],
                        lhsT=vals[:, ks2, sq * P:(sq + 1) * P],
                        rhs=wo_sb[:, ks2, n2 * SQT:(n2 + 1) * SQT],
                        start=(ks2 == 0), stop=(ks2 == 1),
                    )
                ostg = work.tile([P, SQT], bf, tag="ostg", name="ostg")
                nc.vector.tensor_copy(out=ostg[:], in_=ps[:])
                nc.sync.dma_start(
                    out=out[sq * P:(sq + 1) * P, n2 * SQT:(n2 + 1) * SQT],
                    in_=ostg[:])

        for j in range(NJ):
            if j == 0:
                fillers = []
            else:
                fillers = [(lambda sq=(j - 1) * 4 + t: outproj_tile(sq))
                           for t in range(4)]
            attention_col(1, j, fillers)
        for t in range(4):
            outproj_tile((NJ - 1) * 4 + t)

    nc.compile()
    return nc


def get_nc():
    if "nc" not in _CACHE:
        _CACHE["nc"] = _build_nc()
    return _CACHE["nc"]


def make_in_maps(x, Wq, Wk, Wv, Wo, token_positions, rope_theta):
    """Host-side sharding: per-core input dict (bf16, pre-transposed/permuted)."""
    x = np.asarray(x, np.float32)
    Wq = np.asarray(Wq, np.float32)
    Wk = np.asarray(Wk, np.float32)
    Wv = np.asarray(Wv, np.float32)
    Wo = np.asarray(Wo, np.float32)
    pos = np.asarray(token_positions).astype(np.float32)
    theta = float(np.asarray(rope_theta))

    perm = np.concatenate([np.arange(0, DK, 2), np.arange(1, DK, 2)])  # evens, odds
    freqs = theta ** (-np.arange(DK // 2, dtype=np.float32) / (DK // 2))
    ang = pos[:, None] * freqs[None, :]          # [S, 32]
    cosT = np.cos(ang).T.astype(np.float32)      # [32, S]
    sinT = np.sin(ang).T.astype(np.float32)
    cs_t = np.tile(cosT, (4, 1)).astype(BF)                          # [128, S]
    sn_t = np.concatenate([-sinT, sinT, -sinT, sinT], 0).astype(BF)  # [128, S]

    tri_t = np.tril(np.ones((P, P), np.float32)).T.astype(BF)  # keep p<=f
    sigma = np.arange(P)
    sigma = np.where((sigma // 32) % 2 == 0, sigma + 32, sigma - 32)
    swp_t = np.zeros((P, P), np.float32)
    swp_t[sigma, np.arange(P)] = 1.0
    swp_t = swp_t.astype(BF)

    in_maps = []
    for c in range(8):
        b, g = c // 4, c % 4
        hs = slice(g * E, (g + 1) * E)

        def prep_qk(W, scale):
            Wl = W[hs].reshape(HPC, DK, D)[:, perm, :].reshape(E, D) * scale
            return np.ascontiguousarray(Wl.T).astype(BF)

        in_maps.append({
            "xT": np.ascontiguousarray(x[b].T).astype(BF),
            "wq": prep_qk(Wq, 1.0 / np.sqrt(DK)),
            "wk": prep_qk(Wk, 1.0),
            "wv": np.ascontiguousarray(Wv[hs].T).astype(BF),
            "wo": np.ascontiguousarray(Wo[:, hs].T).astype(BF),
            "cs": cs_t, "sn": sn_t, "tri": tri_t, "swp": swp_t,
        })
    return in_maps


def kernel(x, Wq, Wk, Wv, Wo, token_positions, rope_theta):
    nc = get_nc()
    in_maps = make_in_maps(x, Wq, Wk, Wv, Wo, token_positions, rope_theta)
    from concourse.bass_utils import run_bass_kernel_spmd
    r = run_bass_kernel_spmd(nc, in_maps, core_ids=list(range(8)))
    outs = [np.asarray(m["out"], np.float32) for m in r.results]
    full = np.stack([sum(outs[0:4]), sum(outs[4:8])], 0)
    return full.astype(np.float32)


# revision 19
# speedup vs baseline: 1.0379x; 1.0379x over previous
"""Causal multi-head self-attention with RoPE on 8 TRN2 NeuronCores.

Problem (hardcoded): B=2, S=2048, D=1024, H=16, d_k=64, fp32 I/O.

Sharding (data + tensor parallel, per the head-group hint):
  core c in 0..7 -> batch b = c//4, head group g = c%4 (4 heads = 256 dims).
  Wq/Wk/Wv split column-wise (by output head dims), Wo split row-wise.
  Each core computes a partial [S, D] output (bf16); the host sums the 4
  partials per batch in fp32 (row-parallel unshard).

Device kernel layout (engine-balanced software pipeline):
  - Q,K computed transposed [e, s] so scores need no transposes. Weight rows
    are permuted per head (evens then odds) so RoPE becomes full-width
    elementwise ops plus one 32-row-block permutation matmul on the PE.
  - Three overlapped phases, ordered so the PE always has dense work:
      A: per-512-chunk projections of Q/K (head pair 0) + V interleaved with
         the input DMA stream (ACT does PSUM evictions; DVE does RoPE).
      B: attention for head pair 0 with pair-1 projection chunks emitted as
         fillers inside each column, covering ACT-exp and normalize latency.
      C: attention for pair 1, with the previous column's output
         projection + output DMA emitted as in-column fillers.
  - Scores [sk 128, 2(par), sq] per sk-tile in one 2-bank PSUM tile; the two
    per-head score matmuls are adjacent K=64 row-group tiles (tile_position
    auto-derived (0,0)/(64,0)) so they pack concurrently in the PE array; ONE
    wide ACT exp per sk-tile covers both heads. Causally dead sq ranges are
    never computed (matmul, exp and PV all trimmed to [c*128:]). PV runs one
    sk-tile behind scores so column-boundary PSUM WARs never block the queue.
  - Softmax denominator comes free as a 65th output row of the P@V matmul via
    a ones-column appended to V. No max-subtraction (scores bounded; fp32
    PSUM).  All matmul inputs bf16 (full PE rate), fp32 PSUM accumulation.
"""

import numpy as np
import ml_dtypes

B, S, D = 2, 2048, 1024
H, DK = 16, 64
HPC = 4          # heads per core
E = HPC * DK     # 256 output dims per core
P = 128
KS = D // P      # 8 contraction subtiles
SQT = 512        # sq column width
NJ = S // SQT    # 4 sq columns
NSK = S // P     # 16 sk tiles
VW = 128         # per-head V slot: [ones, 63 pad, v0..63]
DK1 = DK + 1
BF = ml_dtypes.bfloat16

_CACHE = {}


def _build_nc():
    import concourse.bacc as bacc
    import concourse.mybir as mybir
    import concourse.tile as tile
    from contextlib import ExitStack

    bf = mybir.dt.bfloat16
    f32 = mybir.dt.float32
    Exp = mybir.ActivationFunctionType.Exp

    nc = bacc.Bacc("TRN2", target_bir_lowering=False)

    xT = nc.dram_tensor("xT", [D, S], bf, kind="ExternalInput")
    wq = nc.dram_tensor("wq", [D, E], bf, kind="ExternalInput")
    wk = nc.dram_tensor("wk", [D, E], bf, kind="ExternalInput")
    wv = nc.dram_tensor("wv", [D, E], bf, kind="ExternalInput")
    wo = nc.dram_tensor("wo", [E, D], bf, kind="ExternalInput")
    cs = nc.dram_tensor("cs", [P, S], bf, kind="ExternalInput")
    sn = nc.dram_tensor("sn", [P, S], bf, kind="ExternalInput")
    tri = nc.dram_tensor("tri", [P, P], bf, kind="ExternalInput")
    swp = nc.dram_tensor("swp", [P, P], bf, kind="ExternalInput")
    out = nc.dram_tensor("out", [S, D], bf, kind="ExternalOutput")

    with tile.TileContext(nc) as tc, ExitStack() as ctx:
        const = ctx.enter_context(tc.tile_pool(name="const", bufs=1))
        work = ctx.enter_context(tc.tile_pool(name="work", bufs=2))
        pexp_pool = ctx.enter_context(tc.tile_pool(name="pexpp", bufs=6))
        mm = ctx.enter_context(tc.tile_pool(name="mm", bufs=2, space="PSUM"))
        stp_pool = ctx.enter_context(tc.tile_pool(name="stp", bufs=2, space="PSUM"))
        pv_pool = ctx.enter_context(tc.tile_pool(name="pvp", bufs=1, space="PSUM"))

        # ---- input DMAs, ordered so phase A can start as early as
        # possible: Wq, x chunk 0, Wk, Wv (V matmuls directly follow q/k of
        # each chunk in the PE queue), RoPE/mask constants, remaining x
        # chunks, Wo last (needed only in phase C).
        xTv = xT.rearrange("(ks p) s -> p ks s", p=P)
        wq_sb = const.tile([P, KS, E], bf, tag="wq")
        nc.sync.dma_start(wq_sb[:], wq.rearrange("(ks p) e -> p ks e", p=P))
        xss = []
        for st in range(NJ):
            xc = const.tile([P, KS, SQT], bf, tag=f"xs{st}", name=f"xs{st}")
            xss.append(xc)
        nc.sync.dma_start(xss[0][:], xTv[:, :, 0:SQT])
        wk_sb = const.tile([P, KS, E], bf, tag="wk")
        nc.sync.dma_start(wk_sb[:], wk.rearrange("(ks p) e -> p ks e", p=P))
        wv_sb = const.tile([P, KS, E], bf, tag="wv")
        nc.sync.dma_start(wv_sb[:], wv.rearrange("(ks p) e -> p ks e", p=P))
        swp_sb = const.tile([P, P], bf, tag="swp")
        nc.sync.dma_start(swp_sb[:], swp[:])
        nc.sync.dma_start(xss[1][:], xTv[:, :, SQT:2 * SQT])
        cs_sb = const.tile([P, S], bf, tag="cs")
        nc.sync.dma_start(cs_sb[:], cs[:])
        sn_sb = const.tile([P, S], bf, tag="sn")
        nc.sync.dma_start(sn_sb[:], sn[:])
        tri_sb = const.tile([P, P], bf, tag="tri")
        nc.sync.dma_start(tri_sb[:], tri[:])
        nc.sync.dma_start(xss[2][:], xTv[:, :, 2 * SQT:3 * SQT])
        nc.sync.dma_start(xss[3][:], xTv[:, :, 3 * SQT:4 * SQT])
        wo_sb = const.tile([P, 2, D], bf, tag="wo")
        nc.sync.dma_start(wo_sb[:], wo.rearrange("(ks p) e -> p ks e", p=P))

        qts = [const.tile([P, S], bf, tag=f"qt{eb}", name=f"qt{eb}") for eb in range(2)]
        kts = [const.tile([P, S], bf, tag=f"kt{eb}", name=f"kt{eb}") for eb in range(2)]
        # V per head padded to 128 cols: [ones, 63 pad, v0..63]. The leading
        # ones column puts the softmax denominator on PSUM partition 0, where
        # DVE can take the reciprocal straight off PSUM (no ACT involvement,
        # which would head-of-line block the exp queue behind a PE wait); the
        # pad keeps the value rows at base partition 64, the only legal base
        # for a >32-partition PSUM read, so normalize is a single DVE mul.
        vaug = const.tile([P, NSK, HPC * VW], bf, tag="vaug")
        vaug4 = vaug.rearrange("p t (h e) -> p t h e", h=HPC)
        nc.vector.memset(vaug4[:, :, :, 1:64], 0.0)
        nc.vector.memset(vaug4[:, :, :, 0], 1.0)
        # normalized attention values, laid out as Wo lhsT [d%128, d//128, sq]
        vals = const.tile([P, 2, S], bf, tag="vals")

        def proj_chunk(w_sb, eb, st, dst, act_evict, c0=0, clen=SQT):
            """One chunk of (W.T @ x.T) for e-block eb, with RoPE applied.
            act_evict: evict main PSUM via ACT (phase A) or DVE (phase B)."""
            sl = slice(st * SQT + c0, st * SQT + c0 + clen)
            ps = mm.tile([P, SQT], f32, tag="mm", name="ps")[:, 0:clen]
            for ks in range(KS):
                nc.tensor.matmul(
                    ps,
                    lhsT=w_sb[:, ks, eb * P:(eb + 1) * P],
                    rhs=xss[st][:, ks, c0:c0 + clen],
                    start=(ks == 0), stop=(ks == KS - 1),
                )
            q0 = work.tile([P, SQT], bf, tag="q0", name="q0")[:, 0:clen]
            if act_evict:
                nc.scalar.copy(out=q0, in_=ps)
            else:
                nc.vector.tensor_copy(out=q0, in_=ps)
            psw = mm.tile([P, SQT], f32, tag="mm", name="psw")[:, 0:clen]
            nc.tensor.matmul(psw, lhsT=swp_sb[:], rhs=q0,
                             start=True, stop=True)
            t = work.tile([P, SQT], bf, tag="ropet", name="t")[:, 0:clen]
            nc.vector.tensor_mul(out=t, in0=q0, in1=cs_sb[:, sl])
            sw = work.tile([P, SQT], bf, tag="ropesw", name="sw")[:, 0:clen]
            nc.vector.tensor_mul(out=sw, in0=psw, in1=sn_sb[:, sl])
            nc.vector.tensor_add(out=dst[:, sl], in0=t, in1=sw)

        # ---- phase A: Q/K pair 0 + V, per x-chunk, following the DMA stream
        for st in range(NJ):
            proj_chunk(wq_sb, 0, st, qts[0], act_evict=True)
            proj_chunk(wk_sb, 0, st, kts[0], act_evict=True)
            # V for all 4 heads, natural [s, e] layout, into vaug slots
            for s4 in range(4):
                sst = st * 4 + s4
                ps = mm.tile([P, SQT], f32, tag="mm")
                pv256 = ps[:, 0:E]
                for ks in range(KS):
                    nc.tensor.matmul(
                        pv256,
                        lhsT=xss[st][:, ks, s4 * P:(s4 + 1) * P],
                        rhs=wv_sb[:, ks, :],
                        start=(ks == 0), stop=(ks == KS - 1),
                    )
                nc.scalar.copy(
                    out=vaug4[:, sst, :, 64:VW],
                    in_=pv256.rearrange("p (h e) -> p h e", h=HPC),
                )

        def attention_col(pair, j, fillers):
            """Attention for head pair `pair`, sq column j. `fillers` are
            independent emission pieces spread between sk-tiles so the PE
            queue never drains on ACT/normalize latency."""
            jsl = slice(j * SQT, (j + 1) * SQT)
            last_i = 4 * j + 3
            n_i = last_i + 1
            fill_at = {}
            for k in range(len(fillers)):
                pos = (n_i * (k + 1)) // (len(fillers) + 1)
                fill_at.setdefault(pos, []).append(k)
            pvts = [pv_pool.tile([P, SQT], f32, tag=f"pv{par}", name=f"pv{par}")
                    for par in range(2)]
            pending = None  # PV runs one sk-tile behind scores/exp

            def emit_pv(item):
                i, pexp, off = item
                for par in range(2):
                    hl = 2 * pair + par
                    nc.tensor.matmul(
                        pvts[par][0:VW, off:SQT],
                        lhsT=vaug[:, i, hl * VW:(hl + 1) * VW],
                        rhs=pexp[:, par, off:SQT],
                        start=(i == 0), stop=(i == last_i),
                    )

            for i in range(n_i):
                c = i - 4 * j
                off = c * P if c > 0 else 0
                stp = stp_pool.tile([P, 2, SQT], f32, tag="stp", name="stp")
                for par in range(2):
                    nc.tensor.matmul(
                        stp[:, par, off:SQT],
                        lhsT=kts[pair][64 * par:64 * par + 64, i * P:(i + 1) * P],
                        rhs=qts[pair][64 * par:64 * par + 64,
                                      j * SQT + off:(j + 1) * SQT],
                        start=True, stop=True,
                    )
                pexp = pexp_pool.tile([P, 2, SQT], bf, tag="pexp", name="pexp")
                nc.scalar.activation(out=pexp[:, :, off:SQT],
                                     in_=stp[:, :, off:SQT], func=Exp)
                if c >= 0:  # diagonal subtile: triangular mask
                    for par in range(2):
                        dsl = pexp[:, par, off:off + P]
                        nc.vector.tensor_mul(out=dsl, in0=dsl, in1=tri_sb[:])
                if pending is not None:
                    emit_pv(pending)
                pending = (i, pexp, off)
                for k in fill_at.get(i, ()):
                    fillers[k]()
            emit_pv(pending)
            # normalize by softmax denominator (PSUM row 0 of pvt): fast
            # reciprocal straight off PSUM on DVE, broadcast, scale, then an
            # SBUF->SBUF DMA into the vals row block for this head.
            for par in range(2):
                rsb = work.tile([1, SQT], f32, tag="rsb")
                nc.vector.reciprocal_approx_fast(out=rsb[0:1, :],
                                                 in_=pvts[par][0:1, :])
                rb = work.tile([VW, SQT], f32, tag="rb")
                nc.gpsimd.partition_broadcast(rb[0:VW, :], rsb[0:1, :],
                                              channels=VW)
                stg = work.tile([P, SQT], bf, tag="stg")
                nc.vector.tensor_mul(out=stg[64:VW, :], in0=pvts[par][64:VW, :],
                                     in1=rb[64:VW, :])
                dst = vals[64 * par:64 * par + 64, pair, jsl]
                nc.sync.dma_start(out=dst, in_=stg[64:VW, :])

        # ---- phase B: attention pair 0, pair-1 projections as fillers
        for j in range(NJ):
            attention_col(0, j, [
                (lambda st=j: proj_chunk(wq_sb, 1, st, qts[1], act_evict=False)),
                (lambda st=j: proj_chunk(wk_sb, 1, st, kts[1], act_evict=False)),
            ])

        # ---- phase C: attention pair 1, previous column's out-projection
        # emitted as fillers of the current column
        def outproj_tile(sq):
            # out[sq*128:(sq+1)*128, :] = vals[:, :, sq-tile].T @ woT
            for n2 in range(2):
                ps = mm.tile([P, SQT], f32, tag="mm")
                for ks2 in range(2):
                    nc.tensor.matmul(
                        ps[:],
                        lhsT=vals[:, ks2, sq * P:(sq + 1) * P],
                        rhs=wo_sb[:, ks2, n2 * SQT:(n2 + 1) * SQT],
                        start=(ks2 == 0), stop=(ks2 == 1),
                    )
                ostg = work.tile([P, SQT], bf, tag="ostg", name="ostg")
                nc.vector.tensor_copy(out=ostg[:], in_=ps[:])
                nc.sync.dma_start(
                    out=out[sq * P:(sq + 1) * P, n2 * SQT:(n2 + 1) * SQT],
                    in_=ostg[:])

        for j in range(NJ):
            if j == 0:
                fillers = []
            else:
                fillers = [(lambda sq=(j - 1) * 4 + t: outproj_tile(sq))
                           for t in range(4)]
            attention_col(1, j, fillers)
        for t in range(4):
            outproj_tile((NJ - 1) * 4 + t)

    nc.compile()
    return nc


def get_nc():
    if "nc" not in _CACHE:
        _CACHE["nc"] = _build_nc()
    return _CACHE["nc"]


def make_in_maps(x, Wq, Wk, Wv, Wo, token_positions, rope_theta):
    """Host-side sharding: per-core input dict (bf16, pre-transposed/permuted)."""
    x = np.asarray(x, np.float32)
    Wq = np.asarray(Wq, np.float32)
    Wk = np.asarray(Wk, np.float32)
    Wv = np.asarray(Wv, np.float32)
    Wo = np.asarray(Wo, np.float32)
    pos = np.asarray(token_positions).astype(np.float32)
    theta = float(np.asarray(rope_theta))

    perm = np.concatenate([np.arange(0, DK, 2), np.arange(1, DK, 2)])  # evens, odds
    freqs = theta ** (-np.arange(DK // 2, dtype=np.float32) / (DK // 2))
    ang = pos[:, None] * freqs[None, :]          # [S, 32]
    cosT = np.cos(ang).T.astype(np.float32)      # [32, S]
    sinT = np.sin(ang).T.astype(np.float32)
    cs_t = np.tile(cosT, (4, 1)).astype(BF)                          # [128, S]
    sn_t = np.concatenate([-sinT, sinT, -sinT, sinT], 0).astype(BF)  # [128, S]

    tri_t = np.tril(np.ones((P, P), np.float32)).T.astype(BF)  # keep p<=f
    sigma = np.arange(P)
    sigma = np.where((sigma // 32) % 2 == 0, sigma + 32, sigma - 32)
    swp_t = np.zeros((P, P), np.float32)
    swp_t[sigma, np.arange(P)] = 1.0
    swp_t = swp_t.astype(BF)

    in_maps = []
    for c in range(8):
        b, g = c // 4, c % 4
        hs = slice(g * E, (g + 1) * E)

        def prep_qk(W, scale):
            Wl = W[hs].reshape(HPC, DK, D)[:, perm, :].reshape(E, D) * scale
            return np.ascontiguousarray(Wl.T).astype(BF)

        in_maps.append({
            "xT": np.ascontiguousarray(x[b].T).astype(BF),
            "wq": prep_qk(Wq, 1.0 / np.sqrt(DK)),
            "wk": prep_qk(Wk, 1.0),
            "wv": np.ascontiguousarray(Wv[hs].T).astype(BF),
            "wo": np.ascontiguousarray(Wo[:, hs].T).astype(BF),
            "cs": cs_t, "sn": sn_t, "tri": tri_t, "swp": swp_t,
        })
    return in_maps


def kernel(x, Wq, Wk, Wv, Wo, token_positions, rope_theta):
    nc = get_nc()
    in_maps = make_in_maps(x, Wq, Wk, Wv, Wo, token_positions, rope_theta)
    from concourse.bass_utils import run_bass_kernel_spmd
    r = run_bass_kernel_spmd(nc, in_maps, core_ids=list(range(8)))
    outs = [np.asarray(m["out"], np.float32) for m in r.results]
    full = np.stack([sum(outs[0:4]), sum(outs[4:8])], 0)
    return full.astype(np.float32)
